# revision 18
# baseline (speedup 1.0000x reference)
"""Trainium2 Bass kernel for ClassificationKNNLoss (N=8192, D=256, K=16, 100 classes).

Strategy (8 cores, data-parallel over rows of the distance matrix):
  - Each core computes a [1024, 8192] block of pairwise distances via the Gram
    trick: psum = x_i . x_j - 0.5*||x_j||^2 (float32r matmuls, K=256 split in
    two 128-chunks + one K=1 norm-row matmul), d = sqrt(||x_i||^2 - 2*psum).
    The diagonal is killed by an extra identity-matmul adding -1e30.
  - ScalarE computes es = exp(SHIFT - d) into fp16 with a free accumulate that
    yields the softmax denominator per row.
  - The label-match bit is packed into the LSB of the fp16 es value; the DVE
    max8 instruction takes per-1024-column top-8 candidates (64/row), and the
    full top-16 (global + matched subsets) is resolved on the candidate
    arrays in a batched pass.  d of selected neighbors = SHIFT - ln(es).
  - Per-row result: row_mean = ln_sum/cnt - ln(denom_shifted) (SHIFT cancels).
    Host sums across rows/cores: loss = -sum(row_mean)/N.

Per-core SPMD trick: every core sees its columns ROTATED by -core*1024 so its
own diagonal block always sits at local columns [r*128, (r+1)*128) of column
group 0 -- one program serves all cores; all core-dependence lives in inputs.
"""
import sys

sys.path.insert(0, "/opt/trn_rl_repo")

import numpy as np

N, D, K, NCORES = 8192, 256, 16, 8
RPC = N // NCORES          # rows per core
RT = RPC // 128            # row-tiles per core (8)
SHIFT = 24.0
NEGBIG = -1.0e30

_PROG = None


def _build_program():
    import concourse.bacc as bacc
    import concourse.mybir as mybir
    from concourse.tile import TileContext

    f32 = mybir.dt.float32
    f32r = mybir.dt.float32r
    f16 = mybir.dt.float16
    u16 = mybir.dt.uint16
    AF = mybir.ActivationFunctionType
    OP = mybir.AluOpType

    nc = bacc.Bacc()

    XT = nc.declare_dram_parameter("xt", [D, N], f32r, isOutput=False)
    NRM = nc.declare_dram_parameter("nrm", [1, N], f32r, isOutput=False)
    YB = nc.declare_dram_parameter("yb", [128, N], f16, isOutput=False)
    YP = nc.declare_dram_parameter("yp", [128, RT], f32, isOutput=False)
    SQN = nc.declare_dram_parameter("sqn", [128, RT], f32, isOutput=False)
    IDI = nc.declare_dram_parameter("idi", [128, 128], f32r, isOutput=False)
    DGR = nc.declare_dram_parameter("dgr", [128, 2048], f32r, isOutput=False)
    ONES = nc.declare_dram_parameter("ones", [1, 128], f32r, isOutput=False)
    RM = nc.declare_dram_parameter("rm", [128, RT], f32, isOutput=True)

    with TileContext(nc) as tc:
        with (
            tc.tile_pool(name="const", bufs=1) as cpool,
            tc.tile_pool(name="es", bufs=2) as espool,
            tc.tile_pool(name="eqv", bufs=1) as eqvpool,
            tc.tile_pool(name="dti", bufs=1) as dpool,
            tc.tile_pool(name="sm", bufs=1) as smpool,
            tc.tile_pool(name="ps", bufs=4, space="PSUM") as pspool,
        ):
            # small resident tiles first (cheap DMAs, needed early)
            nrm = cpool.tile([1, N], f32r, tag="nrm")
            nc.sync.dma_start(out=nrm, in_=NRM[:, :])
            sqn = cpool.tile([128, RT], f32, tag="sqn")
            nc.sync.dma_start(out=sqn, in_=SQN[:, :])
            idi = cpool.tile([128, 128], f32r, tag="idi")
            nc.sync.dma_start(out=idi, in_=IDI[:, :])
            dgr = cpool.tile([128, 2048], f32r, tag="dgr")
            nc.sync.dma_start(out=dgr, in_=DGR[:, :])
            ones = cpool.tile([1, 128], f32r, tag="ones")
            nc.sync.dma_start(out=ones, in_=ONES[:, :])
            shiftc = cpool.tile([128, 1], f32, tag="shiftc")
            nc.vector.memset(shiftc, float(SHIFT))

            # xt blocks in first-use order: both K-halves of column block 0 first
            xt = [[None] * 4 for _ in range(2)]
            for cb in range(4):
                for kc in range(2):
                    t = cpool.tile([128, 2048], f32r, tag=f"xt{kc}{cb}")
                    nc.sync.dma_start(
                        out=t, in_=XT[kc * 128:(kc + 1) * 128, cb * 2048:(cb + 1) * 2048]
                    )
                    xt[kc][cb] = t
            yp = cpool.tile([128, RT], f32, tag="yp")
            nc.sync.dma_start(out=yp, in_=YP[:, :])
            yb = cpool.tile([128, N], f16, tag="yb")
            nc.sync.dma_start(out=yb, in_=YB[:, :])

            # accumulators / batched-final tiles
            cnt = smpool.tile([128, RT], f32, tag="cnt")
            lns = smpool.tile([128, RT], f32, tag="lns")
            dnr = smpool.tile([128, RT], f32, tag="dnr")
            candall = smpool.tile([128, 64 * RT], f16, tag="candall")
            CF = 64 * RT
            lsbm = smpool.tile([128, CF], u16, tag="lsbm")
            cm = smpool.tile([128, CF], f16, tag="cm")
            m1 = smpool.tile([128, 8 * RT], f16, tag="m1")
            m2 = smpool.tile([128, 8 * RT], f16, tag="m2")
            mmall = smpool.tile([128, 16 * RT], f16, tag="mmall")

            for r in range(RT):
                es16 = espool.tile([128, N], f16, tag="es16")
                eqt = eqvpool.tile([128, N], u16, tag="eqt")
                dti = dpool.tile([128, N], f32, tag="dti")

                for cg in range(8):
                    ps = pspool.tile([128, 1024], f32, tag="ps")
                    for cc in range(2):
                        c0 = cg * 1024 + cc * 512
                        oap = ps[:, cc * 512:(cc + 1) * 512]
                        is_diag = (cg == 0 and cc == (r // 4))
                        cb, co = c0 // 2048, c0 % 2048
                        nc.tensor.matmul(
                            out=oap,
                            lhsT=xt[0][0][:, r * 128:(r + 1) * 128],
                            rhs=xt[0][cb][:, co:co + 512],
                            start=True, stop=False,
                        )
                        nc.tensor.matmul(
                            out=oap,
                            lhsT=xt[1][0][:, r * 128:(r + 1) * 128],
                            rhs=xt[1][cb][:, co:co + 512],
                            start=False, stop=False,
                        )
                        if is_diag:
                            nc.tensor.matmul(
                                out=oap, lhsT=idi[:, :],
                                rhs=dgr[:, (r % 4) * 512:(r % 4 + 1) * 512],
                                start=False, stop=False,
                            )
                        nc.tensor.matmul(
                            out=oap,
                            lhsT=ones[:, :],
                            rhs=nrm[:, c0:c0 + 512],
                            start=False, stop=True,
                        )
                    nc.scalar.activation(
                        out=dti[:, cg * 1024:(cg + 1) * 1024], in_=ps, func=AF.Sqrt,
                        scale=-2.0, bias=sqn[:, r:r + 1],
                    )
                dnm = smpool.tile([128, 1], f32, tag=f"dnm{r}")
                nc.scalar.activation(
                    out=es16, in_=dti, func=AF.Exp, scale=-1.0, bias=shiftc[:, :],
                    accum_out=dnm,
                )
                nc.vector.tensor_copy(dnr[:, r:r + 1], dnm)

                # match mask on Pool (hides in the or->or window), packing on DVE
                nc.gpsimd.tensor_scalar(
                    out=eqt, in0=yb, scalar1=yp[:, r:r + 1], scalar2=None,
                    op0=OP.is_equal,
                )
                vt = es16.bitcast(u16)
                nc.vector.tensor_scalar(
                    out=vt, in0=vt, scalar1=0xFFFE, scalar2=None,
                    op0=OP.bitwise_and,
                )
                nc.vector.tensor_tensor(out=vt, in0=vt, in1=eqt, op=OP.bitwise_or)

                for c in range(8):
                    nc.vector.max(
                        out=candall[:, r * 64 + c * 8:r * 64 + (c + 1) * 8],
                        in_=es16[:, c * 1024:(c + 1) * 1024],
                    )

                # per-tile selection chain on the small candidate array
                ca = candall[:, r * 64:(r + 1) * 64]
                nc.vector.tensor_scalar(
                    out=lsbm[:, r * 64:(r + 1) * 64], in0=ca.bitcast(u16),
                    scalar1=1, scalar2=None, op0=OP.bitwise_and,
                )
                cmr = cm[:, r * 64:(r + 1) * 64]
                nc.vector.memset(cmr, -1.0)
                nc.vector.copy_predicated(
                    out=cmr, mask=lsbm[:, r * 64:(r + 1) * 64], data=ca
                )
                nc.vector.max(out=m1[:, r * 8:(r + 1) * 8], in_=ca)
                nc.vector.match_replace(
                    out=ca, in_to_replace=m1[:, r * 8:(r + 1) * 8],
                    in_values=ca, imm_value=-1.0,
                )
                nc.vector.max(out=m2[:, r * 8:(r + 1) * 8], in_=ca)
                nc.vector.max(out=mmall[:, r * 16:r * 16 + 8], in_=cmr)
                nc.vector.match_replace(
                    out=cmr, in_to_replace=mmall[:, r * 16:r * 16 + 8],
                    in_values=cmr, imm_value=-1.0,
                )
                nc.vector.max(
                    out=mmall[:, r * 16 + 8:(r + 1) * 16],
                    in_=cmr,
                )

            # ---- batched threshold/stat finals ----

            # per-tile 16th-largest threshold, cleared LSB, as fp16
            t16c = smpool.tile([128, RT], u16, tag="t16c")
            nc.vector.tensor_scalar(
                out=t16c, in0=m2.bitcast(u16)[:, 7::8], scalar1=0xFFFE,
                scalar2=None, op0=OP.bitwise_and,
            )
            # selm = (mm >= t16) per tile, via broadcast tensor_tensor
            selm = smpool.tile([128, RT, 16], u16, tag="selm")
            nc.vector.tensor_tensor(
                out=selm[:, :, :],
                in0=mmall[:, :].rearrange("p (r k) -> p r k", k=16),
                in1=t16c.bitcast(f16)[:, :].unsqueeze(2).to_broadcast([128, RT, 16]),
                op=OP.is_ge,
            )
            nc.vector.reduce_sum(out=cnt, in_=selm[:, :, :], axis=mybir.AxisListType.X)
            mmsel = smpool.tile([128, 16 * RT], f16, tag="mmsel")
            nc.vector.memset(mmsel, 1.0)
            nc.vector.copy_predicated(
                out=mmsel, mask=selm[:, :, :].rearrange("p r k -> p (r k)"), data=mmall
            )
            lnall = smpool.tile([128, 16 * RT], f32, tag="lnall")
            nc.scalar.activation(out=lnall, in_=mmsel, func=AF.Ln)
            nc.vector.reduce_sum(
                out=lns, in_=lnall[:, :].rearrange("p (r k) -> p r k", k=16),
                axis=mybir.AxisListType.X,
            )

            # row_mean = lns/cnt - ln(dnr), 0 where cnt==0
            lnden = smpool.tile([128, RT], f32, tag="lnden")
            nc.scalar.activation(out=lnden, in_=dnr, func=AF.Ln)
            cntc = smpool.tile([128, RT], f32, tag="cntc")
            nc.vector.tensor_scalar(out=cntc, in0=cnt, scalar1=1.0, scalar2=None, op0=OP.max)
            rcp = smpool.tile([128, RT], f32, tag="rcp")
            nc.vector.reciprocal(out=rcp, in_=cntc)
            t1 = smpool.tile([128, RT], f32, tag="t1")
            nc.vector.tensor_tensor(out=t1, in0=lns, in1=rcp, op=OP.mult)
            nc.vector.tensor_tensor(out=t1, in0=t1, in1=lnden, op=OP.subtract)
            cmask = smpool.tile([128, RT], f32, tag="cmask")
            nc.vector.tensor_scalar(out=cmask, in0=cnt, scalar1=0.5, scalar2=None, op0=OP.is_ge)
            rmt = smpool.tile([128, RT], f32, tag="rmt")
            nc.vector.tensor_tensor(out=rmt, in0=t1, in1=cmask, op=OP.mult)
            nc.sync.dma_start(out=RM[:, :], in_=rmt)

    nc.compile()
    return nc


def _round_f32r(a):
    """Round to hi+lo bf16 pair (exactly representable in PE float32r mode)."""
    import ml_dtypes
    a = np.asarray(a, dtype=np.float32)
    hi = a.astype(ml_dtypes.bfloat16).astype(np.float32)
    lo = (a - hi).astype(ml_dtypes.bfloat16).astype(np.float32)
    return hi + lo


def _host_inputs(x, y):
    y16 = y.astype(np.float16)
    sqn_full = np.einsum("nd,nd->n", x.astype(np.float64), x.astype(np.float64)).astype(np.float32)
    xt_full = _round_f32r(np.ascontiguousarray(x.T))          # [D, N]
    nrm_full = _round_f32r(-0.5 * sqn_full)[None, :]          # [1, N]
    idi_h = np.eye(128, dtype=np.float32)
    dgr_h = np.zeros((128, 2048), dtype=np.float32)
    for v in range(4):
        dgr_h[:, v * 512 + v * 128: v * 512 + (v + 1) * 128] = np.eye(128, dtype=np.float32) * NEGBIG
    ones_h = np.ones((1, 128), dtype=np.float32)

    in_maps = []
    for c in range(NCORES):
        sh = c * RPC
        rows = sh + np.arange(RPC)
        in_maps.append({
            "xt": np.ascontiguousarray(np.roll(xt_full, -sh, axis=1)),
            "nrm": np.ascontiguousarray(np.roll(nrm_full, -sh, axis=1)),
            "yb": np.ascontiguousarray(np.broadcast_to(np.roll(y16, -sh)[None, :], (128, N))),
            "yp": np.ascontiguousarray(y16[rows].reshape(RT, 128).T.astype(np.float32)),
            "sqn": np.ascontiguousarray(sqn_full[rows].reshape(RT, 128).T),
            "idi": idi_h, "dgr": dgr_h, "ones": ones_h,
        })
    return in_maps


def kernel(x, y):
    global _PROG
    from concourse.bass_utils import run_bass_kernel_spmd

    x = np.asarray(x, dtype=np.float32)
    y_in = np.asarray(y)

    if _PROG is None:
        _PROG = _build_program()
    nc = _PROG

    in_maps = _host_inputs(x, y_in)
    res = run_bass_kernel_spmd(nc, in_maps, list(range(NCORES)))
    total = np.float64(0.0)
    for c in range(NCORES):
        total += np.float64(res.results[c]["rm"].astype(np.float64).sum())
    loss = -(total / N)
    return np.float32(loss)
